# revision 32
# baseline (speedup 1.0000x reference)
"""GCN-II style graph convolution on 8 Trainium2 NeuronCores (Bass/Tile).

Computes: out = (1-alpha) * segment_sum(x[adj_col] * adj_val, adj_row, N)
               + alpha * feature

Strategy (fully data-parallel, no collectives):
  - Destination nodes sharded 8 ways; x replicated in every core's DRAM
    (f16). Host-side index preprocessing partitions each core's edges by
    (dest sub-block of 64, source window of 25000); within each group,
    slots are sorted by source so gather reads walk HBM mostly forward.
  - The critical path is dma_gather descriptor generation: the Q7 SWDGE
    workers process ~9ns/row per queue, 4 queues in parallel (~2.2ns/row
    aggregate); the dma_gather *instruction* is just an async enqueue
    that blocks only when its queue is still busy. Hence: one call per
    (super-block of 14 sub-blocks, window) -> 64 calls rotating over 4
    queues, sized for smooth pipelining, plus 4 tiny warm-up gathers at
    the top to absorb the one-time ~14us Q7 init, and tapered final
    super-blocks so the post-gather tail is short.
  - Dest sub-blocks of 64 (not 128) halve the scatter-matrix volume the
    DVE must build (the second engine load, ~1.1ns/elem with a broadcast
    operand) while keeping the same padded-slot count: ~102 edges per
    (sub-block, window) still round to one 128-slot chunk.
  - Scatter matrices built per super-block in TWO fused DVE
    scalar_tensor_tensor ops over an [slot, dest64, chunk] layout:
        t  = ld - iota_d          (ld broadcast over dest, iota real)
        S  = (t == 0) * val       (val broadcast over dest)
  - Per chunk: matmul(ps_half[d,f] += S_c^T(strided stationary) @ xg_c)
    where ps_half is the top/bottom 64 partitions of a shared [128,128]
    PSUM tile (even/odd sub-block), so PSUM-evac stays one ACT copy per
    128 dest rows.
  - alpha*feature enters the same accumulation as matmul(alpha*I64, ft);
    feat loads and out stores are batched per super-block (one DMA each).
"""

import sys

import numpy as np

_TRN_REPO = "/opt/trn_rl_repo"
if _TRN_REPO not in sys.path:
    sys.path.insert(0, _TRN_REPO)

P = 128   # partitions / chunk size (slots per chunk)
B64 = 64  # dest sub-block width
NCORES = 8
SBLK = 7         # 128-row dest blocks per super-block (98 = 14*7)
NQUEUES = 4      # SWDGE queues for gathers
MAXGATHER = 4096  # rows per dma_gather call (split guard)
NWIN = 4         # source windows (int16 gather index reach is 32768)
SINGLE_PACKET = False

F16 = np.float16


def _cdiv(a, b):
    return -(-a // b)


def _sb_sizes(nblk):
    """Uniform SBLK-sized super-blocks (a tapered tail measured worse:
    extra super-blocks add pipeline stalls that outweigh the shorter
    post-gather tail)."""
    sizes = []
    left = nblk
    while left > 0:
        sizes.append(min(SBLK, left))
        left -= sizes[-1]
    return sizes


def _preprocess(x, feature, adj_row, adj_col, adj_val, alpha,
                n_cores=NCORES):
    """Index-only preprocessing: per-core edge partitioning + padding."""
    N, D = x.shape
    E = adj_row.shape[0]
    npc = _cdiv(N, n_cores)          # nodes per core
    nblk = _cdiv(npc, P)             # 128-dest blocks per core
    npad = nblk * P
    nb64 = npad // B64               # 64-dest sub-blocks per core
    sb_sizes = _sb_sizes(nblk)       # per-super-block 128-row block counts
    nsb = len(sb_sizes)
    wwidth = _cdiv(N, NWIN)

    core = adj_row // npc
    d = adj_row - core * npc         # dest local to core
    b = d // B64                     # dest sub-block
    ld = (d % B64).astype(np.float32)
    s = np.minimum(adj_col // wwidth, NWIN - 1)  # source window

    flat = ((core.astype(np.int64) * nb64 + b) * NWIN + s)
    counts = np.bincount(flat, minlength=n_cores * nb64 * NWIN)
    counts = counts.reshape(n_cores, nb64, NWIN)
    nch = _cdiv(counts.max(axis=0), P)           # [nb64, NWIN] chunks

    # slot layout: (super-block, window, sub-block, chunk)-major so each
    # (super-block, window) is one contiguous gather; each super-block's
    # chunk count padded to even (DVE 4B-alignment)
    slot_off = np.zeros((nb64, NWIN), dtype=np.int64)
    gathers = []        # per sb: list of (win, slot_start, n_slots)
    sb_chunk0 = []      # first global chunk of each super-block
    sb_nchunks = []     # chunks per super-block (even)
    sb_b0 = np.concatenate(([0], np.cumsum(sb_sizes))) * 2  # b64 bounds
    off = 0
    for isb in range(nsb):
        blocks = range(sb_b0[isb], sb_b0[isb + 1])
        sb_chunk0.append(off // P)
        calls = []
        for ss in range(NWIN):
            start = off
            for bb in blocks:
                slot_off[bb, ss] = off
                off += int(nch[bb, ss]) * P
            if ss == NWIN - 1 and ((off // P) - sb_chunk0[-1]) % 2 == 1:
                off += P        # pad chunk (gathers row 0 of window, val 0)
            lo = start
            while lo < off:
                n = min(off - lo, MAXGATHER)
                calls.append((ss, lo, n))
                lo += n
        gathers.append(calls)
        sb_nchunks.append((off // P) - sb_chunk0[-1])
    totslot = off
    ctot = totslot // P
    cmax = max(sb_nchunks)

    # scatter each core's edges into its padded slot layout
    idx16 = np.zeros((n_cores, totslot), dtype=np.int16)  # pad: row 0 of win
    ldv = np.zeros((n_cores, totslot), dtype=np.float32)
    valv = np.zeros((n_cores, totslot), dtype=np.float32)  # pad: weight 0

    order = np.argsort(flat, kind="stable")
    fo = flat[order]
    _, first_idx, grp_cnt = np.unique(fo, return_index=True,
                                      return_counts=True)
    rank = np.arange(E, dtype=np.int64) - np.repeat(first_idx, grp_cnt)
    k_s = (fo // (nb64 * NWIN)).astype(np.int64)
    bs = fo % (nb64 * NWIN)
    b_s = (bs // NWIN).astype(np.int64)
    s_s = (bs % NWIN).astype(np.int64)
    # within each (core, sub-block, window) run, order slots by source so
    # the gather's HBM reads walk the window mostly sequentially
    order = order[np.lexsort((adj_col[order], fo))]
    fo = flat[order]
    rank = np.arange(E, dtype=np.int64) - np.repeat(first_idx, grp_cnt)
    pos = slot_off[b_s, s_s] + rank
    idx16[k_s, pos] = (adj_col[order] - s_s * wwidth).astype(np.int16)
    ldv[k_s, pos] = ld[order]
    valv[k_s, pos] = adj_val[order] * (1.0 - alpha)

    # gather-index tile: idx i of a call -> (partition i%16, col i//16),
    # replicated across the 8 groups of 16 partitions. Call offsets are
    # multiples of 128, so one global wrap equals per-call wraps.
    idx_tile = np.ascontiguousarray(
        np.tile(idx16.reshape(n_cores, totslot // 16, 16).transpose(0, 2, 1),
                (1, 8, 1)))
    # chunk-major metadata: column = chunk, partition = edge within chunk
    ld_tile = np.ascontiguousarray(
        ldv.reshape(n_cores, ctot, P).transpose(0, 2, 1)).astype(F16)
    val_tile = np.ascontiguousarray(
        valv.reshape(n_cores, ctot, P).transpose(0, 2, 1)).astype(F16)

    feat_pad = np.zeros((n_cores, npad, D), dtype=F16)
    for k in range(n_cores):
        lo = k * npc
        hi = min(lo + npc, N)
        feat_pad[k, : hi - lo] = feature[lo:hi].astype(F16)

    # iota over dest64, materialized [P, 64 dests, cmax chunks] so every
    # DVE operand has innermost stride 1
    iota_dc = np.broadcast_to(
        np.arange(B64, dtype=np.float32)[None, :, None], (P, B64, cmax))
    iota_dc = np.ascontiguousarray(iota_dc.reshape(P, B64 * cmax)).astype(F16)
    # two stacked alpha*I64 so either PSUM half reads its own partitions
    alpha_eye = np.ascontiguousarray(np.tile(
        (alpha * np.eye(B64, dtype=np.float32)).astype(F16), (2, 1)))
    widx_zero = np.zeros((P, 1), dtype=np.int16)

    # per-sub-block chunk lists (global chunk indices, slot order),
    # plus the per-window split for the last super-block's window-major
    # matmul emission
    chunks_of_block = [[] for _ in range(nb64)]
    chunks_by_bw = [[[] for _ in range(NWIN)] for _ in range(nb64)]
    for bb in range(nb64):
        for ss in range(NWIN):
            c0 = int(slot_off[bb, ss]) // P
            for j in range(int(nch[bb, ss])):
                chunks_of_block[bb].append(c0 + j)
                chunks_by_bw[bb][ss].append(c0 + j)

    meta = dict(N=N, D=D, n_cores=n_cores, npc=npc, nblk=nblk, npad=npad,
                nb64=nb64, sb_b0=sb_b0, wwidth=wwidth, nsb=nsb,
                totslot=totslot, ctot=ctot, cmax=cmax, gathers=gathers,
                sb_chunk0=sb_chunk0, sb_nchunks=sb_nchunks,
                chunks_of_block=chunks_of_block, chunks_by_bw=chunks_by_bw)
    x_bf = np.ascontiguousarray(x.astype(F16))
    in_maps = []
    for k in range(n_cores):
        in_maps.append({
            "x": x_bf,
            "feat": feat_pad[k],
            "idx16": idx_tile[k],
            "ld": ld_tile[k],
            "val": val_tile[k],
            "iotadc": iota_dc,
            "alphaI": alpha_eye,
            "widx": widx_zero,
        })
    return meta, in_maps


def _build(meta):
    """Build + compile the (single, SPMD) Bass program."""
    from contextlib import ExitStack

    import concourse.bacc as bacc
    import concourse.mybir as mybir
    import concourse.tile as tile

    N, D = meta["N"], meta["D"]
    nblk = meta["nblk"]
    nb64 = meta["nb64"]
    sb_b0 = meta["sb_b0"]
    nsb = meta["nsb"]
    wwidth = meta["wwidth"]
    npad = meta["npad"]
    totslot = meta["totslot"]
    ctot = meta["ctot"]
    cmax = meta["cmax"]
    gathers = meta["gathers"]
    sb_chunk0 = meta["sb_chunk0"]
    sb_nchunks = meta["sb_nchunks"]
    chunks_of_block = meta["chunks_of_block"]

    f32 = mybir.dt.float32
    f16 = mybir.dt.float16
    alu = mybir.AluOpType
    nc = bacc.Bacc("TRN2", target_bir_lowering=False, debug=False,
                   num_swdge_queues=NQUEUES)

    x_t = nc.dram_tensor("x", [N, D], f16, kind="ExternalInput").ap()
    feat_t = nc.dram_tensor("feat", [npad, D], f16,
                            kind="ExternalInput").ap()
    idx_t = nc.dram_tensor("idx16", [P, totslot // 16], mybir.dt.int16,
                           kind="ExternalInput").ap()
    ld_t = nc.dram_tensor("ld", [P, ctot], f16, kind="ExternalInput").ap()
    val_t = nc.dram_tensor("val", [P, ctot], f16, kind="ExternalInput").ap()
    iota_t = nc.dram_tensor("iotadc", [P, B64 * cmax], f16,
                            kind="ExternalInput").ap()
    aI_t = nc.dram_tensor("alphaI", [P, B64], f16, kind="ExternalInput").ap()
    widx_t = nc.dram_tensor("widx", [P, 1], mybir.dt.int16,
                            kind="ExternalInput").ap()
    out_t = nc.dram_tensor("out", [npad, D], f32, kind="ExternalOutput").ap()

    with tile.TileContext(nc) as tc, ExitStack() as ctx:
        const = ctx.enter_context(tc.tile_pool(name="const", bufs=1))
        # warm the Q7 SWDGE path on every queue with tiny gathers so the
        # ~14us one-time init overlaps the constant loads
        widx_s = const.tile([P, 1], mybir.dt.int16, name="widx")
        nc.sync.dma_start(widx_s[:], widx_t[:, :])
        warm_s = const.tile([P, NQUEUES, D], f16, name="warm")
        for wq in range(NQUEUES):
            nc.gpsimd.dma_gather(warm_s[:, wq:wq + 1, :], x_t[0:wwidth, :],
                                 widx_s[:, 0:1], 16, 16, D, queue_num=wq,
                                 single_packet=SINGLE_PACKET)
        # load gather indices first, one tile per super-block, so the
        # first gathers start as soon as their own slice lands
        idx_sbs = []
        for isb in range(nsb):
            a = sb_chunk0[isb] * (P // 16)
            bcol = (sb_chunk0[isb] + sb_nchunks[isb]) * (P // 16)
            t = const.tile([P, max(bcol - a, 1)], mybir.dt.int16,
                           name=f"idxsb{isb}", tag=f"idxsb{isb}")
            if bcol > a:
                nc.sync.dma_start(t[:], idx_t[:, a:bcol])
            idx_sbs.append(t)
        iota_s = const.tile([P, B64, cmax], f16)
        nc.sync.dma_start(iota_s[:], iota_t.rearrange("p (d c) -> p d c",
                                                      c=cmax))
        aI_s = const.tile([P, B64], f16)
        nc.sync.dma_start(aI_s[:], aI_t[:, :])
        ld_s = const.tile([P, ctot], f16)
        nc.sync.dma_start(ld_s[:], ld_t[:, :])
        val_s = const.tile([P, ctot], f16)
        nc.sync.dma_start(val_s[:], val_t[:, :])

        xg_pool = ctx.enter_context(tc.tile_pool(name="xg", bufs=4))
        sv_pool = ctx.enter_context(tc.tile_pool(name="sv", bufs=2))
        feat_pool = ctx.enter_context(tc.tile_pool(name="ft", bufs=2))
        psum_pool = ctx.enter_context(
            tc.tile_pool(name="ps", bufs=8, space="PSUM"))
        out_pool = ctx.enter_context(tc.tile_pool(name="ob", bufs=2))

        q = 0
        for isb in range(nsb):
            csb = sb_nchunks[isb]
            c0 = sb_chunk0[isb]
            nb_sub = sb_b0[isb + 1] - sb_b0[isb]
            nb128 = nb_sub // 2
            r0 = sb_b0[isb] * B64
            xg = xg_pool.tile([P, cmax, D], f16, tag="xg")
            for (ss, slot_start, n_slots) in gathers[isb]:
                o = slot_start // P - c0
                win_lo = ss * wwidth
                win_hi = min(win_lo + wwidth, N) if ss < NWIN - 1 else N
                ia = slot_start // 16 - c0 * (P // 16)
                nc.gpsimd.dma_gather(
                    xg[:, o:o + n_slots // P, :],
                    x_t[win_lo:win_hi, :],
                    idx_sbs[isb][:, ia: ia + n_slots // 16],
                    n_slots,
                    n_slots,
                    D,
                    queue_num=q,
                    single_packet=SINGLE_PACKET,
                )
                q = (q + 1) % NQUEUES

            ft = feat_pool.tile([P, SBLK, D], f16, tag="ft")
            nc.sync.dma_start(
                ft[:, :nb128, :],
                feat_t[r0:r0 + nb128 * P, :].rearrange("(b d) f -> d b f",
                                                       d=P))

            # scatter matrices for the whole super-block, two fused DVE
            # ops; [P slot, 64 dest, csb chunk] layout keeps innermost
            # stride 1 on every operand
            sv = sv_pool.tile([P, B64, cmax], f16, tag="sv")
            ld_bc = ld_s[:, None, c0:c0 + csb].to_broadcast([P, B64, csb])
            val_bc = val_s[:, None, c0:c0 + csb].to_broadcast([P, B64, csb])
            nc.vector.scalar_tensor_tensor(
                out=sv[:, :, :csb], in0=ld_bc, scalar=0.0,
                in1=iota_s[:, :, :csb],
                op0=alu.bypass, op1=alu.subtract)
            nc.vector.scalar_tensor_tensor(
                out=sv[:, :, :csb], in0=sv[:, :, :csb], scalar=0.0,
                in1=val_bc,
                op0=alu.is_equal, op1=alu.mult)

            ob = out_pool.tile([P, SBLK, D], f32, tag="ob")
            ps = None
            for j64 in range(nb_sub):
                bb = sb_b0[isb] + j64
                bi, half = j64 // 2, j64 % 2
                if half == 0:
                    ps = psum_pool.tile([P, D], f32, tag="ps")
                lo, hi = half * B64, (half + 1) * B64
                pslice = ps[lo:hi, :]
                chunks = chunks_of_block[bb]
                nc.tensor.matmul(pslice, aI_s[lo:hi, :], ft[lo:hi, bi, :],
                                 start=True, stop=(len(chunks) == 0))
                for i, g in enumerate(chunks):
                    lc = g - c0
                    nc.tensor.matmul(pslice, sv[:, :, lc], xg[:, lc, :],
                                     start=False, stop=(i == len(chunks) - 1))
                if half == 1:
                    nc.scalar.copy(ob[:, bi, :], ps[:])
            nc.sync.dma_start(
                out_t[r0:r0 + nb128 * P, :].rearrange("(b d) f -> d b f",
                                                      d=P),
                ob[:, :nb128, :])

    nc.compile()
    return nc


_CACHE = {}


def _execute(inputs, trace=False, n_cores=NCORES):
    from concourse.bass_utils import run_bass_kernel_spmd

    x = np.asarray(inputs["x"], dtype=np.float32)
    feature = np.asarray(inputs["feature"], dtype=np.float32)
    adj_row = np.asarray(inputs["adj_row"], dtype=np.int64)
    adj_col = np.asarray(inputs["adj_col"], dtype=np.int64)
    adj_val = np.asarray(inputs["adj_val"], dtype=np.float32)
    alpha = float(np.asarray(inputs["alpha"]))

    import hashlib
    h = hashlib.sha256()
    for a in (adj_row, adj_col, adj_val):
        h.update(np.ascontiguousarray(a).tobytes())
    h.update(np.float64(alpha).tobytes())
    key = (x.shape, feature.shape, n_cores, h.hexdigest())

    if key in _CACHE:
        nc, meta = _CACHE[key]
        _, in_maps = _preprocess(x, feature, adj_row, adj_col, adj_val,
                                 alpha, n_cores)
    else:
        meta, in_maps = _preprocess(x, feature, adj_row, adj_col, adj_val,
                                    alpha, n_cores)
        nc = _build(meta)
        _CACHE[key] = (nc, meta)

    res = run_bass_kernel_spmd(nc, in_maps, core_ids=list(range(n_cores)),
                               trace=trace)
    npc = meta["npc"]
    N = meta["N"]
    pieces = []
    for k in range(n_cores):
        lo = k * npc
        hi = min(lo + npc, N)
        pieces.append(res.results[k]["out"][: hi - lo])
    out = np.concatenate(pieces, axis=0).astype(np.float32)
    return out, res


def kernel(**inputs):
    out, _ = _execute(inputs, trace=False)
    return out


# revision 38
# speedup vs baseline: 1.1549x; 1.1549x over previous
"""GCN-II style graph convolution on 8 Trainium2 NeuronCores (Bass/Tile).

Computes: out = (1-alpha) * segment_sum(x[adj_col] * adj_val, adj_row, N)
               + alpha * feature

Strategy (fully data-parallel, no collectives):
  - Destination nodes sharded 8 ways; x replicated in every core's DRAM
    (f16). Host-side index preprocessing partitions each core's edges by
    (dest sub-block of 64, source window of 25000); within each group,
    slots are sorted by source so gather reads walk HBM mostly forward.
  - The critical path is dma_gather descriptor generation: the Q7 SWDGE
    workers process ~9ns/row per queue, 4 queues in parallel (~2.2ns/row
    aggregate); the dma_gather *instruction* is just an async enqueue
    that blocks only when its queue is still busy. Hence: one call per
    (super-block of 14 sub-blocks, window) -> 64 calls rotating over 4
    queues, sized for smooth pipelining, plus 4 tiny warm-up gathers at
    the top to absorb the one-time ~14us Q7 init, and tapered final
    super-blocks so the post-gather tail is short.
  - Dest sub-blocks of 64 (not 128) halve the scatter-matrix volume the
    DVE must build (the second engine load, ~1.1ns/elem with a broadcast
    operand) while keeping the same padded-slot count: ~102 edges per
    (sub-block, window) still round to one 128-slot chunk.
  - Scatter matrices built per super-block in TWO fused DVE
    scalar_tensor_tensor ops over an [slot, dest64, chunk] layout:
        t  = ld - iota_d          (ld broadcast over dest, iota real)
        S  = (t == 0) * val       (val broadcast over dest)
  - Per chunk: matmul(ps_half[d,f] += S_c^T(strided stationary) @ xg_c)
    where ps_half is the top/bottom 64 partitions of a shared [128,128]
    PSUM tile (even/odd sub-block), so PSUM-evac stays one ACT copy per
    128 dest rows.
  - alpha*feature enters the same accumulation as matmul(alpha*I64, ft);
    feat loads and out stores are batched per super-block (one DMA each).
"""

import sys

import numpy as np

_TRN_REPO = "/opt/trn_rl_repo"
if _TRN_REPO not in sys.path:
    sys.path.insert(0, _TRN_REPO)

P = 128   # partitions / chunk size (slots per chunk)
B64 = 64  # dest sub-block width
NCORES = 8
SBLK = 7         # 128-row dest blocks per super-block (98 = 14*7)
NQUEUES = 4      # SWDGE queues for gathers
MAXGATHER = 4096  # rows per dma_gather call (split guard)
NWIN = 4         # source windows (int16 gather index reach is 32768)
SINGLE_PACKET = False

F16 = np.float16


def _cdiv(a, b):
    return -(-a // b)


def _sb_sizes(nblk):
    """Uniform SBLK-sized super-blocks (a tapered tail measured worse:
    extra super-blocks add pipeline stalls that outweigh the shorter
    post-gather tail)."""
    sizes = []
    left = nblk
    while left > 0:
        sizes.append(min(SBLK, left))
        left -= sizes[-1]
    return sizes


def _preprocess(x, feature, adj_row, adj_col, adj_val, alpha,
                n_cores=NCORES):
    """Index-only preprocessing: per-core edge partitioning + padding."""
    N, D = x.shape
    E = adj_row.shape[0]
    npc = _cdiv(N, n_cores)          # nodes per core
    nblk = _cdiv(npc, P)             # 128-dest blocks per core
    npad = nblk * P
    nb64 = npad // B64               # 64-dest sub-blocks per core
    sb_sizes = _sb_sizes(nblk)       # per-super-block 128-row block counts
    nsb = len(sb_sizes)
    wwidth = _cdiv(N, NWIN)

    core = adj_row // npc
    d = adj_row - core * npc         # dest local to core
    b = d // B64                     # dest sub-block
    ld = (d % B64).astype(np.float32)
    s = np.minimum(adj_col // wwidth, NWIN - 1)  # source window

    flat = ((core.astype(np.int64) * nb64 + b) * NWIN + s)
    counts = np.bincount(flat, minlength=n_cores * nb64 * NWIN)
    counts = counts.reshape(n_cores, nb64, NWIN)
    nch = _cdiv(counts.max(axis=0), P)           # [nb64, NWIN] chunks

    # slot layout: (super-block, window, sub-block, chunk)-major so each
    # (super-block, window) is one contiguous gather; each super-block's
    # chunk count padded to even (DVE 4B-alignment)
    slot_off = np.zeros((nb64, NWIN), dtype=np.int64)
    gathers = []        # per sb: list of (win, slot_start, n_slots)
    sb_chunk0 = []      # first global chunk of each super-block
    sb_nchunks = []     # chunks per super-block (even)
    sb_b0 = np.concatenate(([0], np.cumsum(sb_sizes))) * 2  # b64 bounds
    off = 0
    for isb in range(nsb):
        blocks = range(sb_b0[isb], sb_b0[isb + 1])
        sb_chunk0.append(off // P)
        calls = []
        for ss in range(NWIN):
            start = off
            for bb in blocks:
                slot_off[bb, ss] = off
                off += int(nch[bb, ss]) * P
            if ss == NWIN - 1 and ((off // P) - sb_chunk0[-1]) % 2 == 1:
                off += P        # pad chunk (gathers row 0 of window, val 0)
            lo = start
            while lo < off:
                n = min(off - lo, MAXGATHER)
                calls.append((ss, lo, n))
                lo += n
        gathers.append(calls)
        sb_nchunks.append((off // P) - sb_chunk0[-1])
    totslot = off
    ctot = totslot // P
    cmax = max(sb_nchunks)

    # scatter each core's edges into its padded slot layout
    idx16 = np.zeros((n_cores, totslot), dtype=np.int16)  # pad: row 0 of win
    ldv = np.zeros((n_cores, totslot), dtype=np.float32)
    valv = np.zeros((n_cores, totslot), dtype=np.float32)  # pad: weight 0

    order = np.argsort(flat, kind="stable")
    fo = flat[order]
    _, first_idx, grp_cnt = np.unique(fo, return_index=True,
                                      return_counts=True)
    rank = np.arange(E, dtype=np.int64) - np.repeat(first_idx, grp_cnt)
    k_s = (fo // (nb64 * NWIN)).astype(np.int64)
    bs = fo % (nb64 * NWIN)
    b_s = (bs // NWIN).astype(np.int64)
    s_s = (bs % NWIN).astype(np.int64)
    # within each (core, sub-block, window) run, order slots by source so
    # the gather's HBM reads walk the window mostly sequentially
    order = order[np.lexsort((adj_col[order], fo))]
    fo = flat[order]
    rank = np.arange(E, dtype=np.int64) - np.repeat(first_idx, grp_cnt)
    pos = slot_off[b_s, s_s] + rank
    idx16[k_s, pos] = (adj_col[order] - s_s * wwidth).astype(np.int16)
    ldv[k_s, pos] = ld[order]
    valv[k_s, pos] = adj_val[order] * (1.0 - alpha)

    # gather-index tile: idx i of a call -> (partition i%16, col i//16),
    # replicated across the 8 groups of 16 partitions. Call offsets are
    # multiples of 128, so one global wrap equals per-call wraps.
    idx_tile = np.ascontiguousarray(
        np.tile(idx16.reshape(n_cores, totslot // 16, 16).transpose(0, 2, 1),
                (1, 8, 1)))
    # chunk-major metadata: column = chunk, partition = edge within chunk
    ld_tile = np.ascontiguousarray(
        ldv.reshape(n_cores, ctot, P).transpose(0, 2, 1)).astype(F16)
    val_tile = np.ascontiguousarray(
        valv.reshape(n_cores, ctot, P).transpose(0, 2, 1)).astype(F16)

    feat_pad = np.zeros((n_cores, npad, D), dtype=F16)
    for k in range(n_cores):
        lo = k * npc
        hi = min(lo + npc, N)
        feat_pad[k, : hi - lo] = feature[lo:hi].astype(F16)

    # iota over dest64, materialized [P, 64 dests, cmax chunks] so every
    # DVE operand has innermost stride 1
    iota_dc = np.broadcast_to(
        np.arange(B64, dtype=np.float32)[None, :, None], (P, B64, cmax))
    iota_dc = np.ascontiguousarray(iota_dc.reshape(P, B64 * cmax)).astype(F16)
    # two stacked alpha*I64 so either PSUM half reads its own partitions
    alpha_eye = np.ascontiguousarray(np.tile(
        (alpha * np.eye(B64, dtype=np.float32)).astype(F16), (2, 1)))
    widx_zero = np.zeros((P, 1), dtype=np.int16)

    # per-sub-block chunk lists (global chunk indices, slot order),
    # plus the per-window split for the last super-block's window-major
    # matmul emission
    chunks_of_block = [[] for _ in range(nb64)]
    chunks_by_bw = [[[] for _ in range(NWIN)] for _ in range(nb64)]
    for bb in range(nb64):
        for ss in range(NWIN):
            c0 = int(slot_off[bb, ss]) // P
            for j in range(int(nch[bb, ss])):
                chunks_of_block[bb].append(c0 + j)
                chunks_by_bw[bb][ss].append(c0 + j)

    meta = dict(N=N, D=D, n_cores=n_cores, npc=npc, nblk=nblk, npad=npad,
                nb64=nb64, sb_b0=sb_b0, wwidth=wwidth, nsb=nsb,
                totslot=totslot, ctot=ctot, cmax=cmax, gathers=gathers,
                sb_chunk0=sb_chunk0, sb_nchunks=sb_nchunks,
                chunks_of_block=chunks_of_block, chunks_by_bw=chunks_by_bw)
    x_bf = np.ascontiguousarray(x.astype(F16))
    in_maps = []
    for k in range(n_cores):
        in_maps.append({
            "x": x_bf,
            "feat": feat_pad[k],
            "idx16": idx_tile[k],
            "ld": ld_tile[k],
            "val": val_tile[k],
            "iotadc": iota_dc,
            "alphaI": alpha_eye,
            "widx": widx_zero,
        })
    return meta, in_maps


def _build(meta):
    """Build + compile the (single, SPMD) Bass program."""
    from contextlib import ExitStack

    import concourse.bacc as bacc
    import concourse.mybir as mybir
    import concourse.tile as tile

    N, D = meta["N"], meta["D"]
    nblk = meta["nblk"]
    nb64 = meta["nb64"]
    sb_b0 = meta["sb_b0"]
    nsb = meta["nsb"]
    wwidth = meta["wwidth"]
    npad = meta["npad"]
    totslot = meta["totslot"]
    ctot = meta["ctot"]
    cmax = meta["cmax"]
    gathers = meta["gathers"]
    sb_chunk0 = meta["sb_chunk0"]
    sb_nchunks = meta["sb_nchunks"]
    chunks_of_block = meta["chunks_of_block"]
    chunks_by_bw = meta["chunks_by_bw"]

    f32 = mybir.dt.float32
    f16 = mybir.dt.float16
    alu = mybir.AluOpType
    nc = bacc.Bacc("TRN2", target_bir_lowering=False, debug=False,
                   num_swdge_queues=NQUEUES)

    x_t = nc.dram_tensor("x", [N, D], f16, kind="ExternalInput").ap()
    feat_t = nc.dram_tensor("feat", [npad, D], f16,
                            kind="ExternalInput").ap()
    idx_t = nc.dram_tensor("idx16", [P, totslot // 16], mybir.dt.int16,
                           kind="ExternalInput").ap()
    ld_t = nc.dram_tensor("ld", [P, ctot], f16, kind="ExternalInput").ap()
    val_t = nc.dram_tensor("val", [P, ctot], f16, kind="ExternalInput").ap()
    iota_t = nc.dram_tensor("iotadc", [P, B64 * cmax], f16,
                            kind="ExternalInput").ap()
    aI_t = nc.dram_tensor("alphaI", [P, B64], f16, kind="ExternalInput").ap()
    widx_t = nc.dram_tensor("widx", [P, 1], mybir.dt.int16,
                            kind="ExternalInput").ap()
    out_t = nc.dram_tensor("out", [npad, D], f32, kind="ExternalOutput").ap()

    with tile.TileContext(nc) as tc, ExitStack() as ctx:
        const = ctx.enter_context(tc.tile_pool(name="const", bufs=1))
        # warm the Q7 SWDGE path on every queue with tiny gathers so the
        # ~14us one-time init overlaps the constant loads
        widx_s = const.tile([P, 1], mybir.dt.int16, name="widx")
        nc.sync.dma_start(widx_s[:], widx_t[:, :])
        warm_s = const.tile([P, NQUEUES, D], f16, name="warm")
        for wq in range(NQUEUES):
            nc.gpsimd.dma_gather(warm_s[:, wq:wq + 1, :], x_t[0:wwidth, :],
                                 widx_s[:, 0:1], 16, 16, D, queue_num=wq,
                                 single_packet=SINGLE_PACKET)
        # load gather indices first, one tile per super-block, so the
        # first gathers start as soon as their own slice lands
        idx_sbs = []
        for isb in range(nsb):
            a = sb_chunk0[isb] * (P // 16)
            bcol = (sb_chunk0[isb] + sb_nchunks[isb]) * (P // 16)
            t = const.tile([P, max(bcol - a, 1)], mybir.dt.int16,
                           name=f"idxsb{isb}", tag=f"idxsb{isb}")
            if bcol > a:
                nc.sync.dma_start(t[:], idx_t[:, a:bcol])
            idx_sbs.append(t)
        iota_s = const.tile([P, B64, cmax], f16)
        nc.sync.dma_start(iota_s[:], iota_t.rearrange("p (d c) -> p d c",
                                                      c=cmax))
        aI_s = const.tile([P, B64], f16)
        nc.sync.dma_start(aI_s[:], aI_t[:, :])
        ld_s = const.tile([P, ctot], f16)
        nc.sync.dma_start(ld_s[:], ld_t[:, :])
        val_s = const.tile([P, ctot], f16)
        nc.sync.dma_start(val_s[:], val_t[:, :])

        xg_pool = ctx.enter_context(tc.tile_pool(name="xg", bufs=4))
        sv_pool = ctx.enter_context(tc.tile_pool(name="sv", bufs=2))
        feat_pool = ctx.enter_context(tc.tile_pool(name="ft", bufs=2))
        psum_pool = ctx.enter_context(
            tc.tile_pool(name="ps", bufs=8, space="PSUM"))
        out_pool = ctx.enter_context(tc.tile_pool(name="ob", bufs=2))

        q = 0
        for isb in range(nsb):
            csb = sb_nchunks[isb]
            c0 = sb_chunk0[isb]
            nb_sub = sb_b0[isb + 1] - sb_b0[isb]
            nb128 = nb_sub // 2
            r0 = sb_b0[isb] * B64
            xg = xg_pool.tile([P, cmax, D], f16, tag="xg")
            for (ss, slot_start, n_slots) in gathers[isb]:
                o = slot_start // P - c0
                win_lo = ss * wwidth
                win_hi = min(win_lo + wwidth, N) if ss < NWIN - 1 else N
                ia = slot_start // 16 - c0 * (P // 16)
                nc.gpsimd.dma_gather(
                    xg[:, o:o + n_slots // P, :],
                    x_t[win_lo:win_hi, :],
                    idx_sbs[isb][:, ia: ia + n_slots // 16],
                    n_slots,
                    n_slots,
                    D,
                    queue_num=q,
                    single_packet=SINGLE_PACKET,
                )
                q = (q + 1) % NQUEUES

            ft = feat_pool.tile([P, SBLK, D], f16, tag="ft")
            nc.sync.dma_start(
                ft[:, :nb128, :],
                feat_t[r0:r0 + nb128 * P, :].rearrange("(b d) f -> d b f",
                                                       d=P))

            # scatter matrices for the whole super-block, two fused DVE
            # ops; [P slot, 64 dest, csb chunk] layout keeps innermost
            # stride 1 on every operand
            sv = sv_pool.tile([P, B64, cmax], f16, tag="sv")
            ld_bc = ld_s[:, None, c0:c0 + csb].to_broadcast([P, B64, csb])
            val_bc = val_s[:, None, c0:c0 + csb].to_broadcast([P, B64, csb])
            nc.vector.scalar_tensor_tensor(
                out=sv[:, :, :csb], in0=ld_bc, scalar=0.0,
                in1=iota_s[:, :, :csb],
                op0=alu.bypass, op1=alu.subtract)
            nc.vector.scalar_tensor_tensor(
                out=sv[:, :, :csb], in0=sv[:, :, :csb], scalar=0.0,
                in1=val_bc,
                op0=alu.is_equal, op1=alu.mult)

            ob = out_pool.tile([P, SBLK, D], f32, tag="ob")
            if isb < nsb - 1:
                ps = None
                for j64 in range(nb_sub):
                    bb = sb_b0[isb] + j64
                    bi, half = j64 // 2, j64 % 2
                    if half == 0:
                        ps = psum_pool.tile([P, D], f32, tag="ps")
                    lo, hi = half * B64, (half + 1) * B64
                    pslice = ps[lo:hi, :]
                    chunks = chunks_of_block[bb]
                    nc.tensor.matmul(pslice, aI_s[lo:hi, :], ft[lo:hi, bi, :],
                                     start=True, stop=(len(chunks) == 0))
                    for i, g in enumerate(chunks):
                        lc = g - c0
                        nc.tensor.matmul(pslice, sv[:, :, lc], xg[:, lc, :],
                                         start=False,
                                         stop=(i == len(chunks) - 1))
                    if half == 1:
                        nc.scalar.copy(ob[:, bi, :], ps[:])
            else:
                # last super-block: window-major so each gather call's
                # matmuls issue as soon as that call lands -- only the
                # final window's work remains after the last gather
                pss = [psum_pool.tile([P, D], f32, name=f"psl{bi}",
                                      tag="ps")
                       for bi in range(nb128)]
                left = {}
                for j64 in range(nb_sub):
                    bb = sb_b0[isb] + j64
                    bi, half = j64 // 2, j64 % 2
                    lo, hi = half * B64, (half + 1) * B64
                    left[bb] = len(chunks_of_block[bb])
                    nc.tensor.matmul(pss[bi][lo:hi, :], aI_s[lo:hi, :],
                                     ft[lo:hi, bi, :], start=True,
                                     stop=(left[bb] == 0))
                for ss in range(NWIN):
                    for j64 in range(nb_sub):
                        bb = sb_b0[isb] + j64
                        bi, half = j64 // 2, j64 % 2
                        lo, hi = half * B64, (half + 1) * B64
                        for g in chunks_by_bw[bb][ss]:
                            left[bb] -= 1
                            nc.tensor.matmul(pss[bi][lo:hi, :],
                                             sv[:, :, g - c0],
                                             xg[:, g - c0, :], start=False,
                                             stop=(left[bb] == 0))
                for bi in range(nb128):
                    nc.scalar.copy(ob[:, bi, :], pss[bi][:])
            nc.sync.dma_start(
                out_t[r0:r0 + nb128 * P, :].rearrange("(b d) f -> d b f",
                                                      d=P),
                ob[:, :nb128, :])

    nc.compile()
    return nc


_CACHE = {}


def _execute(inputs, trace=False, n_cores=NCORES):
    from concourse.bass_utils import run_bass_kernel_spmd

    x = np.asarray(inputs["x"], dtype=np.float32)
    feature = np.asarray(inputs["feature"], dtype=np.float32)
    adj_row = np.asarray(inputs["adj_row"], dtype=np.int64)
    adj_col = np.asarray(inputs["adj_col"], dtype=np.int64)
    adj_val = np.asarray(inputs["adj_val"], dtype=np.float32)
    alpha = float(np.asarray(inputs["alpha"]))

    import hashlib
    h = hashlib.sha256()
    for a in (adj_row, adj_col, adj_val):
        h.update(np.ascontiguousarray(a).tobytes())
    h.update(np.float64(alpha).tobytes())
    key = (x.shape, feature.shape, n_cores, h.hexdigest())

    if key in _CACHE:
        nc, meta = _CACHE[key]
        _, in_maps = _preprocess(x, feature, adj_row, adj_col, adj_val,
                                 alpha, n_cores)
    else:
        meta, in_maps = _preprocess(x, feature, adj_row, adj_col, adj_val,
                                    alpha, n_cores)
        nc = _build(meta)
        _CACHE[key] = (nc, meta)

    res = run_bass_kernel_spmd(nc, in_maps, core_ids=list(range(n_cores)),
                               trace=trace)
    npc = meta["npc"]
    N = meta["N"]
    pieces = []
    for k in range(n_cores):
        lo = k * npc
        hi = min(lo + npc, N)
        pieces.append(res.results[k]["out"][: hi - lo])
    out = np.concatenate(pieces, axis=0).astype(np.float32)
    return out, res


def kernel(**inputs):
    out, _ = _execute(inputs, trace=False)
    return out
